# revision 3
# baseline (speedup 1.0000x reference)
"""Trainium2 Bass kernel v2 for nn_LongThinNet (16-layer thin MLP, batch 2^20).

Data-parallel over 8 cores (131072 rows each). Feature-major compute:
activations live as [feature-partitions, batch-free] bf16 tiles; the 10-wide
layers run as full-width matmuls with block-diagonal bf16 weights (12
batch-interleaved j-slices in 4 x 32-aligned bands of 3 for the "ab" block,
8 j-slices for the "c" block). Per supergroup (16384 rows) one unified
[128, 1536] fp32 PSUM tile (3 banks: ab half0, ab half1, c).

v2 vs baseline:
  - bf16 x / weights / activations / output: halves DMA + SBUF traffic,
    enables FWL fast weight loads; matmul still 1 col/cycle.
  - biases folded into the matmuls via a constant-1 partition (x row 120 at
    L0, partition 30 kept at 1.0 through all layers by a diagonal 1 in the
    weights) -> activations are single-op, bias-free.
  - activation work split between the ACT engine (direct Prelu on the two ab
    banks) and DVE (c bank via u = 0.5*z then max(2u, u)) at PSUM-bank
    granularity; every 5th layer ACT takes all three banks to balance the
    engines. Separate per-supergroup ab/c PSUM pools (c double-buffered) keep
    WAR dependencies fine-grained so the two interleaved supergroups pipeline.
  - L15 feature-major block-diagonal like the middle layers (3 matmuls,
    3 LDWEIGHTS instead of 12), output copied bank-split like an act.
"""

import sys

sys.path.insert(0, "/opt/trn_rl_repo")

from contextlib import ExitStack

import numpy as np
import ml_dtypes

import concourse.bass as bass
import concourse.mybir as mybir
import concourse.tile as tile
from concourse.bass_utils import run_bass_kernel_spmd

F32 = mybir.dt.float32
BF16 = mybir.dt.bfloat16
AF = mybir.ActivationFunctionType
ALU = mybir.AluOpType
NPBF16 = ml_dtypes.bfloat16

NCORES = 8
BC = 131072          # rows per core
IN, HID = 40, 10
NMID = 14            # middle 10->10 layers
SG = 8               # supergroups per core, 16384 rows each

# (band, gamma) layout: ab j-slices 0..11 per half at partition 32*b+10*g;
# c j-slices 24..31 at partitions {0,10,20, 32,42,52, 64,74}
BANDS_AB = [(b, g) for b in range(4) for g in range(3)]
BANDS_C = [(b, g) for b in range(2) for g in range(3)] + [(2, g) for g in range(2)]

# per-j (0..31): partition base and column base inside the [128, 1536] tile
_PJ = np.zeros(32, np.int64)
_CJ = np.zeros(32, np.int64)
for j in range(24):
    h, r = divmod(j, 12)
    k, g = divmod(r, 3)
    _PJ[j] = 32 * k + 10 * g
    _CJ[j] = 512 * h
for j in range(24, 32):
    jj = j - 24
    if jj < 6:
        b, g = divmod(jj, 3)
        _PJ[j] = 32 * b + 10 * g
    else:
        _PJ[j] = 64 + 10 * (jj - 6)
    _CJ[j] = 1024


def _skip(name):
    return name in ("InstEventSemaphore", "InstAllEngineBarrier")


def _split_multi_waits(nc):
    """walrus codegen allows <=1 semaphore wait per instruction; hoist extras
    onto standalone InstEventSemaphore instructions inserted just before."""
    n_new = 0
    for f in nc.m.functions:
        for bb in f.blocks:
            out, changed = [], False
            for inst in bb.instructions:
                si = inst.sync_info
                if si is not None and len(si.on_wait) > 1 and not _skip(type(inst).__name__):
                    waits = list(si.on_wait)
                    for w in waits[:-1]:
                        n_new += 1
                        out.append(
                            mybir.InstEventSemaphore(
                                name=f"EVW-{n_new}-{inst.name}",
                                engine=inst.engine,
                                sync_info=mybir.SyncInfo(on_wait=[w], on_update=[]),
                            )
                        )
                    inst.sync_info = mybir.SyncInfo(
                        on_wait=[waits[-1]], on_update=list(si.on_update)
                    )
                    changed = True
                out.append(inst)
            if changed:
                try:
                    bb.instructions = out
                except Exception:
                    lst = bb.instructions
                    lst.clear()
                    lst.extend(out)
    return n_new


def _pack_weights(W_in, b_in, W_mid, b_mid, W_out, b_out):
    """bf16 block-diagonal weights with biases folded in via the constant-1
    partition: x row 120 feeds L0 biases and initializes psum partition 30 to
    1.0; weights' [30, 30+128*l] diagonal 1 keeps it alive through the mid
    layers; L15 reads it for b_out."""
    # L0 ab: 4 accumulation variants (k = band), each [121, 128]
    wl0a = np.zeros((121, 4 * 128), np.float32)
    for k in range(4):
        for g in range(3):
            c0 = 128 * k + 32 * k + 10 * g
            wl0a[40 * g:40 * g + 40, c0:c0 + 10] = W_in.T
            wl0a[120, c0:c0 + 10] = b_in
    wl0a[120, 30] = 1.0  # k=0 initializes the constant-1 psum row

    # L0 c: 2 accumulation variants [121, 96] (tiles 8,9 -> bands 0,1)
    wl0ca = np.zeros((121, 2 * 96), np.float32)
    for k in range(2):
        for g in range(3):
            c0 = 96 * k + 32 * k + 10 * g
            wl0ca[40 * g:40 * g + 40, c0:c0 + 10] = W_in.T
            wl0ca[120, c0:c0 + 10] = b_in
    wl0ca[120, 30] = 1.0
    # L0 c: third matmul [80, 96] (tile 10 -> band 2), bias came with k=0
    wl0cb = np.zeros((80, 96), np.float32)
    for g in range(2):
        c0 = 64 + 10 * g
        wl0cb[40 * g:40 * g + 40, c0:c0 + 10] = W_in.T

    wmid = np.zeros((128, NMID * 128), np.float32)
    wmidc = np.zeros((84, NMID * 84), np.float32)
    for l in range(NMID):
        for b, g in BANDS_AB:
            q = 32 * b + 10 * g
            wmid[q:q + 10, 128 * l + q:128 * l + q + 10] = W_mid[l].T
            wmid[30, 128 * l + q:128 * l + q + 10] = b_mid[l]
        wmid[30, 128 * l + 30] = 1.0
        for b, g in BANDS_C:
            q = 32 * b + 10 * g
            wmidc[q:q + 10, 84 * l + q:84 * l + q + 10] = W_mid[l].T
            wmidc[30, 84 * l + q:84 * l + q + 10] = b_mid[l]
        wmidc[30, 84 * l + 30] = 1.0

    # L15 feature-major block-diag: output lands at the same partitions/cols
    wl15 = np.zeros((128, 128), np.float32)
    for b, g in BANDS_AB:
        q = 32 * b + 10 * g
        wl15[q:q + 10, q:q + 10] = W_out.T
        wl15[30, q:q + 10] = b_out
    wl15c = np.zeros((84, 84), np.float32)
    for b, g in BANDS_C:
        q = 32 * b + 10 * g
        wl15c[q:q + 10, q:q + 10] = W_out.T
        wl15c[30, q:q + 10] = b_out

    return {k: v.astype(NPBF16) for k, v in
            {"wl0a": wl0a, "wl0ca": wl0ca, "wl0cb": wl0cb,
             "wmid": wmid, "wmidc": wmidc,
             "wl15": wl15, "wl15c": wl15c}.items()}


def _pack_x_core(xc):
    """[131072, 40] f32 -> feature-major bf16 [SG, 121, 11*512]:
    partition 40*gamma+f of col block t holds x[row(g, p, 3t+gamma), f] with
    free index 128*g+p; t=10 is the (j=30,31) pair in rows 0..79; row 120 is
    the constant-1 bias feed."""
    a = xc.reshape(SG, 4, 128, 32, IN).transpose(0, 3, 4, 1, 2)  # [sg,j,f,g,p]
    out = np.zeros((SG, 121, 11, 512), np.float32)
    out[:, :120, :10] = (
        a[:, :30].reshape(SG, 10, 3 * IN, 512).transpose(0, 2, 1, 3)
    )
    out[:, :80, 10] = a[:, 30:32].reshape(SG, 2 * IN, 512)
    out[:, 120, :] = 1.0
    return np.ascontiguousarray(
        out.reshape(SG, 121, 11 * 512)).astype(NPBF16)


def _unpack_out_core(oc):
    """[SG, 128, 1536] -> [131072, 10] float32."""
    oc = np.asarray(oc, np.float32)
    idx_p = _PJ[:, None] + np.arange(HID)[None, :]          # [32, 10]
    gp = np.arange(512)
    idx_c = _CJ[:, None] + gp[None, :]                      # [32, 512]
    # res[sg, j, jf, gp]
    res = oc[:, idx_p[:, :, None], idx_c[:, None, :]]       # [SG, 32, 10, 512]
    # row = sg*16384 + gp*32 + j
    res = res.transpose(0, 3, 1, 2)                          # [SG, gp, j, jf]
    return np.ascontiguousarray(res.reshape(BC, HID))


def _build_nc(reps=1, rot=5):
    nc = bass.Bass("TRN2", target_bir_lowering=False, debug=False)

    x_d = nc.dram_tensor("x", [SG, 121, 11 * 512], BF16, kind="ExternalInput").ap()
    wl0a_d = nc.dram_tensor("wl0a", [121, 512], BF16, kind="ExternalInput").ap()
    wl0ca_d = nc.dram_tensor("wl0ca", [121, 192], BF16, kind="ExternalInput").ap()
    wl0cb_d = nc.dram_tensor("wl0cb", [80, 96], BF16, kind="ExternalInput").ap()
    wmid_d = nc.dram_tensor("wmid", [128, NMID * 128], BF16, kind="ExternalInput").ap()
    wmidc_d = nc.dram_tensor("wmidc", [84, NMID * 84], BF16, kind="ExternalInput").ap()
    wl15_d = nc.dram_tensor("wl15", [128, 128], BF16, kind="ExternalInput").ap()
    wl15c_d = nc.dram_tensor("wl15c", [84, 84], BF16, kind="ExternalInput").ap()
    out_d = nc.dram_tensor("out", [SG, 128, 1536], BF16, kind="ExternalOutput").ap()

    with tile.TileContext(nc) as tc, ExitStack() as ctx:
        sc = ctx.enter_context(tc.tile_pool(name="sc", bufs=1))
        sx = ctx.enter_context(tc.tile_pool(name="sx", bufs=4))
        sh = ctx.enter_context(tc.tile_pool(name="sh", bufs=4))
        so = ctx.enter_context(tc.tile_pool(name="so", bufs=3))
        ppa = [ctx.enter_context(tc.tile_pool(name=f"ppa{s}", bufs=1, space="PSUM"))
               for s in range(2)]
        ppc = [ctx.enter_context(tc.tile_pool(name=f"ppc{s}", bufs=2, space="PSUM"))
               for s in range(2)]

        consts = {}
        _const_specs = [
            ("wl0a", wl0a_d, [121, 512]), ("wl0ca", wl0ca_d, [121, 192]),
            ("wl0cb", wl0cb_d, [80, 96]),
            ("wmid", wmid_d, [128, NMID * 128]), ("wmidc", wmidc_d, [84, NMID * 84]),
            ("wl15", wl15_d, [128, 128]), ("wl15c", wl15c_d, [84, 84]),
        ]

        def _load_consts(names):
            for name, dram, shape in _const_specs:
                if name in names:
                    t = sc.tile(shape, BF16, name=f"c_{name}", tag=name)
                    nc.sync.dma_start(t[:], dram)
                    consts[name] = t

        def act_split(dst_tile, pa, pc, r):
            """Bank-split Prelu: ACT runs Prelu directly on the two ab PSUM
            banks; DVE handles the c bank as u = 0.5*z (walrus forbids a
            dual-PSUM-read, so one PSUM->SBUF op) then max(2u, u) = prelu(z)
            all-SBUF. Every 5th layer (r == 4) ACT takes the c bank too,
            rebalancing ACT (~1.2 cols/ns) vs the 2-op DVE lane (~0.5)."""
            nc.scalar.activation(dst_tile[:, 0:1024], pa[:],
                                 AF.Prelu, scale=1.0, alpha=0.5)
            if r == 4:  # rebalance layer: ACT takes the c bank as well
                nc.scalar.activation(dst_tile[:, 1024:1536], pc[:],
                                     AF.Prelu, scale=1.0, alpha=0.5)
                return
            u = sh.tile([128, 512], BF16, name="u", tag="u")
            nc.vector.tensor_scalar(u[:], pc[:],
                                    0.5, None, ALU.mult, ALU.bypass)
            nc.vector.scalar_tensor_tensor(
                dst_tile[:, 1024:1536], u[:], 2.0, u[:], ALU.mult, ALU.max)

        def copy_split(dst_tile, pa, pc, r):
            nc.vector.tensor_copy(dst_tile[:, 0:1024], pa[:])
            nc.vector.tensor_copy(dst_tile[:, 1024:1536], pc[:])

        loop_ctx = tc.For_i(0, reps, 1) if reps > 1 else None
        if loop_ctx is not None:
            ctx.enter_context(loop_ctx)
        for pair in range(SG // 2):
            sgs = (2 * pair, 2 * pair + 1)
            x_lo, x_hi, s_h = {}, {}, {}
            for s, sg in enumerate(sgs):
                x_lo[s] = sx.tile([121, 6 * 512], BF16, name=f"xlo{s}", tag="xlo")
                x_hi[s] = sx.tile([121, 5 * 512], BF16, name=f"xhi{s}", tag="xhi")
                nc.sync.dma_start(x_lo[s][:], x_d[sg][:, 0:6 * 512])
                nc.sync.dma_start(x_hi[s][:], x_d[sg][:, 6 * 512:11 * 512])
                if pair == 0 and s == 0:
                    _load_consts({"wl0a", "wl0ca", "wl0cb"})
                if pair == 0 and s == 1:
                    _load_consts({"wmid", "wmidc", "wl15", "wl15c"})

            # L0: 40 -> 10, block-diag x3, accumulated into banded psum
            for s in range(2):
                def xsl(t):
                    if t < 6:
                        return x_lo[s][:, 512 * t:512 * t + 512]
                    return x_hi[s][:, 512 * (t - 6):512 * (t - 6) + 512]
                pa = ppa[s].tile([128, 1024], F32, name=f"pa{s}", tag="pa")
                pc = ppc[s].tile([128, 512], F32, name=f"pc{s}", tag="pc")
                for half in range(2):
                    for k in range(4):
                        t = 4 * half + k
                        nc.tensor.matmul(
                            pa[:, 512 * half:512 * half + 512],
                            consts["wl0a"][:, 128 * k:128 * k + 128],
                            xsl(t),
                            start=(k == 0), stop=(k == 3),
                        )
                for k in (0, 1):
                    nc.tensor.matmul(
                        pc[0:96, :], consts["wl0ca"][:, 96 * k:96 * k + 96],
                        xsl(8 + k),
                        start=(k == 0), stop=False,
                    )
                nc.tensor.matmul(
                    pc[0:96, :], consts["wl0cb"][:],
                    x_hi[s][0:80, 512 * 4:512 * 5],
                    start=False, stop=True,
                )
                s_h[s] = sh.tile([128, 1536], BF16, name=f"h{s}", tag=f"h{s}")
                act_split(s_h[s], pa, pc, 4 if (rot and 0 % rot == rot - 1) else 0)

            # 14 middle layers, two supergroups interleaved
            for l in range(NMID):
                wm = consts["wmid"][:, 128 * l:128 * l + 128]
                wmc = consts["wmidc"][0:84, 84 * l:84 * l + 84]
                npa, npc, ns = {}, {}, {}
                for s in range(2):
                    npa[s] = ppa[s].tile([128, 1024], F32, name=f"npa{s}", tag="pa")
                    npc[s] = ppc[s].tile([128, 512], F32, name=f"npc{s}", tag="pc")
                    nc.tensor.matmul(npa[s][:, 0:512], wm,
                                     s_h[s][:, 0:512], start=True, stop=True)
                    nc.tensor.matmul(npa[s][:, 512:1024], wm,
                                     s_h[s][:, 512:1024], start=True, stop=True)
                    nc.tensor.matmul(npc[s][0:84, :], wmc,
                                     s_h[s][0:84, 1024:1536], start=True, stop=True)
                for s in range(2):
                    ns[s] = sh.tile([128, 1536], BF16, name=f"nh{s}", tag=f"h{s}")
                    act_split(ns[s], npa[s], npc[s],
                              4 if (rot and (l + 1) % rot == rot - 1) else 0)
                    s_h[s] = ns[s]

            # L15: feature-major block-diag, bank-split copy out, DMA
            for s, sg in enumerate(sgs):
                pa = ppa[s].tile([128, 1024], F32, name=f"pfa{s}", tag="pa")
                pc = ppc[s].tile([128, 512], F32, name=f"pfc{s}", tag="pc")
                nc.tensor.matmul(pa[:, 0:512], consts["wl15"][:],
                                 s_h[s][:, 0:512], start=True, stop=True)
                nc.tensor.matmul(pa[:, 512:1024], consts["wl15"][:],
                                 s_h[s][:, 512:1024], start=True, stop=True)
                nc.tensor.matmul(pc[0:84, :], consts["wl15c"][0:84, :],
                                 s_h[s][0:84, 1024:1536], start=True, stop=True)
                s_o = so.tile([128, 1536], BF16, name="so", tag="out")
                copy_split(s_o, pa, pc, 0)
                nc.sync.dma_start(out_d[sg], s_o[:])

    _split_multi_waits(nc)
    return nc


_NC_CACHE = {}


def kernel(x, W_in, b_in, W_mid, b_mid, W_out, b_out):
    x = np.asarray(x, np.float32)
    W_in = np.asarray(W_in, np.float32)
    b_in = np.asarray(b_in, np.float32)
    W_mid = np.asarray(W_mid, np.float32)
    b_mid = np.asarray(b_mid, np.float32)
    W_out = np.asarray(W_out, np.float32)
    b_out = np.asarray(b_out, np.float32)

    if "nc" not in _NC_CACHE:
        _NC_CACHE["nc"] = _build_nc()
    nc = _NC_CACHE["nc"]

    consts = _pack_weights(W_in, b_in, W_mid, b_mid, W_out, b_out)

    in_maps = []
    for c in range(NCORES):
        xc = _pack_x_core(x[c * BC:(c + 1) * BC])
        in_maps.append({"x": xc, **consts})

    res = run_bass_kernel_spmd(nc, in_maps, list(range(NCORES)))

    outs = [_unpack_out_core(res.results[c]["out"]) for c in range(NCORES)]
    return np.ascontiguousarray(np.concatenate(outs, axis=0))
